# revision 20
# baseline (speedup 1.0000x reference)
"""Multi-head attention block (B=2, N=2048, C=2560, H=40, D=64) on 8 TRN2 NeuronCores.

Sharding: tensor-parallel over heads — core c owns heads 5c..5c+4 for both
batch elements. Each core computes qkv for its heads (full token range),
attention, and its partial contribution to the output projection; the host
sums the 8 partials and adds the (folded) output bias.

Per-core pipeline (per batch element, 2048 tokens):
  phase Q: stream xT in 256-token chunks; qkT matmuls (fp32r) produce
           Q^T/K^T feature-major tiles (fp16, +bias via DVE tensor_scalar);
           v matmuls produce V token-major tiles (fp16, with a ones column
           per head for the softmax denominator).
  phase A: per head-pair / q-block: S^T = k @ q^T via row-packed (tile_position)
           K=64 fp32->? fp16 matmuls; exp on ACT (scale folded in, no max
           subtraction needed at these magnitudes) -> P^T fp16; O^T and the
           denominator accumulate via [v|1] lhsT (M=65); normalization via
           DVE reciprocal + PE K=1 broadcast + DVE multiply (psum-shift for
           the odd head).
  phase P: y_partial = O^T.T @ w_proj (fp32r), streamed to DRAM.
"""
import numpy as np

import concourse.bacc as bacc
import concourse.mybir as mybir
import concourse.tile as tile
from concourse.bass_utils import run_bass_kernel_spmd

F32R = mybir.dt.float32r
F32 = mybir.dt.float32
F16 = mybir.dt.float16
AF = mybir.ActivationFunctionType

B, N, C = 2, 2048, 2560
H, D = 40, 64
NCORES = 8
HPC = H // NCORES            # 5 heads per core
SCALE = D ** -0.5
TOK = B * N                  # 4096
CK = 256                     # token chunk in phase Q
NCHUNK = N // CK             # 8 chunks per batch
KT16 = N // 128              # 16 k-tiles per batch
QB = 512                     # q-block
NQB = N // QB                # 4 q-blocks
KC = C // 128                # 20 contraction tiles

_CACHE = {}
DEBUG = False


def _build():
    nc = bacc.Bacc("TRN2", target_bir_lowering=False, debug=False, num_devices=NCORES)
    xT_d = nc.dram_tensor("xT", [C, TOK], F32R, kind="ExternalInput")
    wall_d = nc.dram_tensor("wall", [C, 960], F32R, kind="ExternalInput")   # q|k (640) + v (320)
    wp_d = nc.dram_tensor("wp", [384, C], F16, kind="ExternalInput")        # padded 320->384
    bias_d = nc.dram_tensor("bias", [128, 6], F32, kind="ExternalInput")    # per-ft qk bias
    y_d = nc.dram_tensor("y", [TOK, C], F32, kind="ExternalOutput")
    dbg = {}
    if DEBUG:
        dbg["q"] = nc.dram_tensor("dq", [B, 3, 128, N], F32, kind="ExternalOutput")
        dbg["k"] = nc.dram_tensor("dk", [B, 3, 128, N], F32, kind="ExternalOutput")
        dbg["v"] = nc.dram_tensor("dv", [B, KT16, 128, HPC * 65], F32, kind="ExternalOutput")
        dbg["o"] = nc.dram_tensor("do", [B, 3, 128, N], F32, kind="ExternalOutput")

    with tile.TileContext(nc) as tc:
        with (
            tc.tile_pool(name="sb", bufs=1) as pool,
            tc.tile_pool(name="ps", bufs=1, space="PSUM") as ps,
        ):
            ones16 = pool.tile([128, 64], F16, name="ones16")
            nc.vector.memset(ones16[:], 1.0)
            bias_sb = pool.tile([128, 6], F32, name="bias_sb")
            nc.sync.dma_start(out=bias_sb[:], in_=bias_d[:])

            proj_q = []
            ypc = [0]

            def emit_proj(drain=False):
                if not proj_q:
                    return
                t, n, OTq, wpq, boffq = proj_q.pop(0)
                tsl = slice(t * 128, (t + 1) * 128)
                nsl = slice(n * 512, (n + 1) * 512)
                tag = ["ypb", "bcb", "T0", "T1"][ypc[0] % 4] if drain else "ypb"
                ypc[0] += 1
                yp = ps.tile([128, 512], F32, tag=tag, name=f"yp{boffq}_{t}_{n}")
                nc.tensor.matmul(yp[:, 0:512], OTq[0][:, tsl], wpq[:, 0, nsl],
                                 start=True, stop=False)
                nc.tensor.matmul(yp[:, 0:512], OTq[1][:, tsl], wpq[:, 1, nsl],
                                 start=False, stop=False)
                nc.tensor.matmul(yp[:, 0:512], OTq[2][:, tsl], wpq[:, 2, nsl],
                                 start=False, stop=True)
                y_sb = pool.tile([128, 512], F32, tag="y", name=f"ysb{boffq}_{t}_{n}", bufs=3)
                nc.vector.tensor_copy(y_sb[:], yp[:, 0:512])
                nc.sync.dma_start(
                    out=y_d[boffq + t * 128: boffq + (t + 1) * 128, nsl], in_=y_sb[:])

            for b in range(B):
                boff = b * N

                # ---------------- phase Q: qkv projections ----------------
                scope_q = nc.enter_named_scope(f"phQ{b}", False)
                wall = pool.tile([128, KC, 960], F32R, tag="w", name=f"wall{b}")
                wall_r = wall_d.rearrange("(t p) f -> p t f", p=128)
                nc.sync.dma_start(out=wall[:, 0:KC // 2, :], in_=wall_r[:, 0:KC // 2, :])
                nc.sync.dma_start(out=wall[:, KC // 2:KC, :], in_=wall_r[:, KC // 2:KC, :])

                QT = [pool.tile([128, N], F16, tag=f"qt{i}", name=f"QT{i}_{b}") for i in range(3)]
                KT = [pool.tile([128, N], F16, tag=f"kt{i}", name=f"KT{i}_{b}") for i in range(3)]
                V = [pool.tile([128, HPC * 65], F16, tag=f"v{i}", name=f"V{i}_{b}")
                     for i in range(KT16)]
                V3 = [t.rearrange("p (h e) -> p h e", h=HPC) for t in V]
                for t3 in V3:
                    nc.vector.memset(t3[:, :, 64:65], 1.0)

                # NOTE: matmul start=True clears has_written for the WHOLE psum
                # bank, so each bank may hold only ONE accumulation group.
                for j in range(NCHUNK):
                    xb = pool.tile([128, KC, CK], F32R, tag=f"x{j % 2}", name=f"xb{b}_{j}")
                    nc.sync.dma_start(
                        out=xb[:],
                        in_=xT_d[:, boff + j * CK: boff + (j + 1) * CK]
                        .rearrange("(t p) c -> p t c", p=128))
                    cj = slice(j * CK, (j + 1) * CK)
                    tQ = ps.tile([128, 1024], F32, tag="T0", name=f"tQ{b}_{j}")
                    tK = ps.tile([128, 1024], F32, tag="T1", name=f"tK{b}_{j}")
                    tMa = ps.tile([128, 512], F32, tag="bcb", name=f"tMa{b}_{j}")
                    tMb = ps.tile([128, 512], F32, tag="ypb", name=f"tMb{b}_{j}")
                    tV = ps.tile([128, 1024], F32, tag="T2", name=f"tV{b}_{j}")
                    for K in range(KC):
                        st, sp = K == 0, K == KC - 1
                        nc.tensor.matmul(tQ[:, 0:256], wall[:, K, 0:128], xb[:, K, :],
                                         start=st, stop=sp)
                        nc.tensor.matmul(tQ[:, 512:768], wall[:, K, 128:256], xb[:, K, :],
                                         start=st, stop=sp)
                    with nc.allow_low_precision(reason="fp16 qkv"):
                        nc.vector.tensor_scalar_add(QT[0][:, cj], tQ[:, 0:256], bias_sb[:, 0:1])
                        nc.vector.tensor_scalar_add(QT[1][:, cj], tQ[:, 512:768], bias_sb[:, 1:2])
                    for K in range(KC):
                        st, sp = K == 0, K == KC - 1
                        nc.tensor.matmul(tK[:, 0:256], wall[:, K, 320:448], xb[:, K, :],
                                         start=st, stop=sp)
                        nc.tensor.matmul(tK[:, 512:768], wall[:, K, 448:576], xb[:, K, :],
                                         start=st, stop=sp)
                    with nc.allow_low_precision(reason="fp16 qkv"):
                        nc.vector.tensor_scalar_add(KT[0][:, cj], tK[:, 0:256], bias_sb[:, 3:4])
                        nc.vector.tensor_scalar_add(KT[1][:, cj], tK[:, 512:768], bias_sb[:, 4:5])
                    for K in range(KC):
                        st, sp = K == 0, K == KC - 1
                        nc.tensor.matmul(tMa[0:64, 0:256], wall[:, K, 256:320],
                                         xb[:, K, :], start=st, stop=sp, tile_position=(0, 0))
                        nc.tensor.matmul(tMb[0:64, 0:256], wall[:, K, 576:640],
                                         xb[:, K, :], start=st, stop=sp, tile_position=(0, 0))
                    with nc.allow_low_precision(reason="fp16 qkv"):
                        nc.vector.tensor_scalar_add(QT[2][0:64, cj], tMa[0:64, 0:256],
                                                    bias_sb[0:64, 2:3])
                        nc.vector.tensor_scalar_add(KT[2][0:64, cj], tMb[0:64, 0:256],
                                                    bias_sb[0:64, 5:6])
                    for K in range(KC):
                        st, sp = K == 0, K == KC - 1
                        nc.tensor.matmul(tV[:, 0:320], xb[:, K, 0:128], wall[:, K, 640:960],
                                         start=st, stop=sp)
                        nc.tensor.matmul(tV[:, 512:832], xb[:, K, 128:256], wall[:, K, 640:960],
                                         start=st, stop=sp)
                    with nc.allow_low_precision(reason="fp16 qkv"):
                        nc.vector.tensor_copy(
                            V3[2 * j][:, :, 0:64],
                            tV[:, 0:320].rearrange("p (h e) -> p h e", h=HPC))
                        nc.vector.tensor_copy(
                            V3[2 * j + 1][:, :, 0:64],
                            tV[:, 512:832].rearrange("p (h e) -> p h e", h=HPC))

                if DEBUG:
                    for i in range(3):
                        qf = pool.tile([128, N], F32, tag="dbg", name=f"dbq{b}_{i}")
                        nc.scalar.activation(qf[:], QT[i][:], AF.Copy)
                        nc.sync.dma_start(out=dbg["q"][b, i], in_=qf[:])
                        kf = pool.tile([128, N], F32, tag="dbg", name=f"dbk{b}_{i}")
                        nc.scalar.activation(kf[:], KT[i][:], AF.Copy)
                        nc.sync.dma_start(out=dbg["k"][b, i], in_=kf[:])
                    for i in range(KT16):
                        vf = pool.tile([128, N], F32, tag="dbg", name=f"dbv{b}_{i}")
                        nc.scalar.activation(vf[:, 0:HPC * 65], V[i][:], AF.Copy)
                        nc.sync.dma_start(out=dbg["v"][b, i], in_=vf[:, 0:HPC * 65])

                # ------------- phase A + P: attention with interleaved proj -------------
                nc.leave_named_scope(f"phQ{b}", scope_q[0], False)
                scope_a = nc.enter_named_scope(f"phA{b}", False)
                OT = [pool.tile([128, N], F16, tag=f"ot{i}", name=f"OT{i}_{b}") for i in range(3)]
                nc.vector.memset(OT[2][64:128, :], 0.0)
                wp = pool.tile([128, 3, C], F16, tag="wpt", name=f"wp{b}")
                nc.sync.dma_start(out=wp[:], in_=wp_d.rearrange("(g p) f -> p g f", p=128))


                OTb, wpb, boffb = OT, wp, boff

                def normalize(o_ps, r_t, rb_t, cols, dst, qbs, shift, nm):
                    """o_ps [0:65, 512]; divide rows 0:64 by denominator row 64
                    (reciprocal already computed into r_t)."""
                    bcb = ps.tile([128, 512], F32, tag="bcb", name=f"bc{nm}")
                    nc.tensor.matmul(bcb[0:64, :], ones16[64:65, :], r_t[64:65, cols],
                                     start=True, stop=True, tile_position=(64, 0))
                    nc.vector.tensor_copy(rb_t[0:64, cols], bcb[0:64, :])
                    if not shift:
                        with nc.allow_low_precision(reason="o f32r"):
                            nc.vector.tensor_mul(dst[0:64, qbs], o_ps[0:64, :], rb_t[0:64, cols])
                    else:
                        nc.vector.tensor_mul(bcb[64:128, :], o_ps[0:64, :], rb_t[0:64, cols])
                        with nc.allow_low_precision(reason="o f32r"):
                            nc.vector.tensor_copy(dst[64:128, qbs], bcb[64:128, :])

                pending = [None]

                def flush_pending():
                    if pending[0] is not None:
                        pending[0]()
                        pending[0] = None

                def unit_pair(p, qb):
                    """S/exp run one 2-kt step ahead of the O matmuls; the previous
                    unit's deferred normalize is emitted after step 1 so its bcast
                    matmul never head-blocks the PE queue."""
                    qt, kt_, qbs = QT[p], KT[p], slice(qb * QB, (qb + 1) * QB)
                    oAB = ps.tile([128, 1024], F32, tag="T2", name=f"oAB{b}_{p}_{qb}")

                    def s_step(k2):
                        pts = []
                        for kt in (2 * k2, 2 * k2 + 1):
                            s = ps.tile([128, 1024], F32, tag=f"T{kt % 2}",
                                        name=f"s{b}_{p}_{qb}_{kt}")
                            ksl = slice(kt * 128, (kt + 1) * 128)
                            nc.tensor.matmul(s[:, 0:512], kt_[0:64, ksl], qt[0:64, qbs],
                                             start=True, stop=True, tile_position=(0, 0))
                            nc.tensor.matmul(s[:, 512:1024], kt_[64:128, ksl], qt[64:128, qbs],
                                             start=True, stop=True, tile_position=(64, 0))
                            p_t = pool.tile([128, 1024], F16, tag="p",
                                            name=f"p{b}_{p}_{qb}_{kt}", bufs=4)
                            with nc.allow_low_precision(reason="fp16 probs"):
                                nc.scalar.activation(p_t[:], s[:], AF.Exp, scale=SCALE)
                            pts.append(p_t)
                        return pts

                    def o_step(k2, pts):
                        for i, kt in enumerate((2 * k2, 2 * k2 + 1)):
                            st, sp = kt == 0, kt == KT16 - 1
                            nc.tensor.matmul(oAB[0:65, 0:512], V3[kt][:, 2 * p, :],
                                             pts[i][:, 0:512], start=st, stop=sp)
                            nc.tensor.matmul(oAB[0:65, 512:1024], V3[kt][:, 2 * p + 1, :],
                                             pts[i][:, 512:1024], start=st, stop=sp)
                        if k2 % 2 == 0:
                            emit_proj()

                    prev = s_step(0)
                    for k2 in range(1, KT16 // 2):
                        cur = s_step(k2)
                        if k2 == 1:
                            flush_pending()
                        o_step(k2 - 1, prev)
                        prev = cur
                    o_step(KT16 // 2 - 1, prev)
                    r_t = pool.tile([128, 1024], F16, tag="r", name=f"r{b}_{p}_{qb}", bufs=2)
                    with nc.allow_low_precision(reason="softmax recip"):
                        nc.vector.reciprocal(r_t[64:65, 0:512], oAB[64:65, 0:512])
                        nc.vector.reciprocal(r_t[64:65, 512:1024], oAB[64:65, 512:1024])

                    def _norm():
                        rb_t = pool.tile([128, 1024], F32, tag="rb", name=f"rb{b}_{p}_{qb}", bufs=2)
                        normalize(oAB[:, 0:512], r_t, rb_t, slice(0, 512), OT[p], qbs,
                                  False, f"{b}_{p}_{qb}a")
                        normalize(oAB[:, 512:1024], r_t, rb_t, slice(512, 1024), OT[p], qbs,
                                  True, f"{b}_{p}_{qb}b")
                    pending[0] = _norm

                def unit_lone(qb):
                    qbs = slice(qb * QB, (qb + 1) * QB)
                    oC = ps.tile([128, 1024], F32, tag="T2", name=f"oC{b}_{qb}")

                    def s_step(k2):
                        pts = []
                        for kt in (2 * k2, 2 * k2 + 1):
                            s = ps.tile([128, 1024], F32, tag=f"T{kt % 2}", name=f"sl{b}_{qb}_{kt}")
                            ksl = slice(kt * 128, (kt + 1) * 128)
                            nc.tensor.matmul(s[:, 0:512], KT[2][0:64, ksl], QT[2][0:64, qbs],
                                             start=True, stop=True, tile_position=(0, 0))
                            p_t = pool.tile([128, 1024], F16, tag="p",
                                            name=f"pl{b}_{qb}_{kt}", bufs=4)
                            with nc.allow_low_precision(reason="fp16 probs"):
                                nc.scalar.activation(p_t[:, 0:512], s[:, 0:512], AF.Exp, scale=SCALE)
                            pts.append(p_t)
                        return pts

                    def o_step(k2, pts):
                        for i, kt in enumerate((2 * k2, 2 * k2 + 1)):
                            st, sp = kt == 0, kt == KT16 - 1
                            nc.tensor.matmul(oC[0:65, 0:512], V3[kt][:, 4, :], pts[i][:, 0:512],
                                             start=st, stop=sp)
                        emit_proj()
                        emit_proj()

                    prev = s_step(0)
                    for k2 in range(1, KT16 // 2):
                        cur = s_step(k2)
                        if k2 == 1:
                            flush_pending()
                        o_step(k2 - 1, prev)
                        prev = cur
                    o_step(KT16 // 2 - 1, prev)
                    r_t = pool.tile([128, 1024], F16, tag="r", name=f"rl{b}_{qb}", bufs=2)
                    with nc.allow_low_precision(reason="softmax recip"):
                        nc.vector.reciprocal(r_t[64:65, 0:512], oC[64:65, 0:512])

                    def _norm():
                        rb_t = pool.tile([128, 1024], F32, tag="rb", name=f"rbl{b}_{qb}", bufs=2)
                        normalize(oC[:, 0:512], r_t, rb_t, slice(0, 512), OT[2], qbs,
                                  False, f"{b}_l{qb}")
                    pending[0] = _norm

                for qb in range(NQB):
                    unit_pair(0, qb)
                    unit_pair(1, qb)
                    unit_lone(qb)
                    for t in range(qb * 4, qb * 4 + 4):
                        for n in range(5):
                            proj_q.append((t, n, OTb, wpb, boffb))
                flush_pending()

                if DEBUG:
                    for i in range(3):
                        of = pool.tile([128, N], F32, tag="dbg", name=f"dbo{b}_{i}")
                        nc.scalar.activation(of[:], OT[i][:], AF.Copy)
                        nc.sync.dma_start(out=dbg["o"][b, i], in_=of[:])
                nc.leave_named_scope(f"phA{b}", scope_a[0], False)
            while proj_q:
                emit_proj(drain=True)
    return nc


def kernel(x, w_qkv, b_qkv, w_proj, b_proj):
    x = np.asarray(x, np.float32)
    w_qkv = np.asarray(w_qkv, np.float32)
    b_qkv = np.asarray(b_qkv, np.float32)
    w_proj = np.asarray(w_proj, np.float32)
    b_proj = np.asarray(b_proj, np.float32)

    if "nc" not in _CACHE:
        nc = _build()
        nc.compile()
        _CACHE["nc"] = nc
    nc = _CACHE["nc"]

    xT = np.ascontiguousarray(x.reshape(TOK, C).T)            # [C, TOK]
    in_maps = []
    for c in range(NCORES):
        f0 = c * HPC * D                                       # 320*c
        qcols = slice(f0, f0 + HPC * D)
        wq = w_qkv[:, qcols]
        wk = w_qkv[:, C + f0: C + f0 + HPC * D]
        wv = w_qkv[:, 2 * C + f0: 2 * C + f0 + HPC * D]
        wall = np.concatenate([wq, wk, wv], axis=1)            # [C, 960]
        wp = np.zeros((384, C), np.float16)
        wp[0:320] = w_proj[f0:f0 + HPC * D, :]
        bias = np.zeros((128, 6), np.float32)
        bq = b_qkv[qcols]
        bk = b_qkv[C + f0: C + f0 + HPC * D]
        bias[:, 0] = bq[0:128]
        bias[:, 1] = bq[128:256]
        bias[0:64, 2] = bq[256:320]
        bias[:, 3] = bk[0:128]
        bias[:, 4] = bk[128:256]
        bias[0:64, 5] = bk[256:320]
        in_maps.append({"xT": xT, "wall": np.ascontiguousarray(wall),
                        "wp": wp, "bias": bias})

    _CACHE["in_maps"] = in_maps
    res = run_bass_kernel_spmd(nc, in_maps, core_ids=list(range(NCORES)))
    _CACHE["results"] = res.results
    y = np.zeros((TOK, C), np.float64)
    for c in range(NCORES):
        y += res.results[c]["y"].astype(np.float64)
    bias_eff = b_proj + b_qkv[2 * C:] @ w_proj                 # v-bias folded through proj
    y += bias_eff
    return y.reshape(B, N, C).astype(np.float32)


# revision 22
# speedup vs baseline: 1.0089x; 1.0089x over previous
"""Multi-head attention block (B=2, N=2048, C=2560, H=40, D=64) on 8 TRN2 NeuronCores.

Sharding: tensor-parallel over heads — core c owns heads 5c..5c+4 for both
batch elements. Each core computes qkv for its heads (full token range),
attention, and its partial contribution to the output projection; the host
sums the 8 partials and adds the (folded) output bias.

Per-core pipeline (per batch element, 2048 tokens):
  phase Q: stream xT in 256-token chunks; qkT matmuls (fp32r) produce
           Q^T/K^T feature-major tiles (fp16, +bias via DVE tensor_scalar);
           v matmuls produce V token-major tiles (fp16, with a ones column
           per head for the softmax denominator).
  phase A: per head-pair / q-block: S^T = k @ q^T via row-packed (tile_position)
           K=64 fp32->? fp16 matmuls; exp on ACT (scale folded in, no max
           subtraction needed at these magnitudes) -> P^T fp16; O^T and the
           denominator accumulate via [v|1] lhsT (M=65); normalization via
           DVE reciprocal + PE K=1 broadcast + DVE multiply (psum-shift for
           the odd head).
  phase P: y_partial = O^T.T @ w_proj (fp32r), streamed to DRAM.
"""
import numpy as np

import concourse.bacc as bacc
import concourse.mybir as mybir
import concourse.tile as tile
from concourse.bass_utils import run_bass_kernel_spmd

F32R = mybir.dt.float32r
F32 = mybir.dt.float32
F16 = mybir.dt.float16
AF = mybir.ActivationFunctionType

B, N, C = 2, 2048, 2560
H, D = 40, 64
NCORES = 8
HPC = H // NCORES            # 5 heads per core
SCALE = D ** -0.5
TOK = B * N                  # 4096
CK = 256                     # token chunk in phase Q
NCHUNK = N // CK             # 8 chunks per batch
KT16 = N // 128              # 16 k-tiles per batch
QB = 512                     # q-block
NQB = N // QB                # 4 q-blocks
KC = C // 128                # 20 contraction tiles

_CACHE = {}
DEBUG = False


def _build():
    nc = bacc.Bacc("TRN2", target_bir_lowering=False, debug=False, num_devices=NCORES)
    xT_d = nc.dram_tensor("xT", [C, TOK], F32R, kind="ExternalInput")
    wall_d = nc.dram_tensor("wall", [C, 960], F32R, kind="ExternalInput")   # q|k (640) + v (320)
    wp_d = nc.dram_tensor("wp", [384, C], F16, kind="ExternalInput")        # padded 320->384
    bias_d = nc.dram_tensor("bias", [128, 6], F32, kind="ExternalInput")    # per-ft qk bias
    y_d = nc.dram_tensor("y", [TOK, C], F32, kind="ExternalOutput")
    dbg = {}
    if DEBUG:
        dbg["q"] = nc.dram_tensor("dq", [B, 3, 128, N], F32, kind="ExternalOutput")
        dbg["k"] = nc.dram_tensor("dk", [B, 3, 128, N], F32, kind="ExternalOutput")
        dbg["v"] = nc.dram_tensor("dv", [B, KT16, 128, HPC * 65], F32, kind="ExternalOutput")
        dbg["o"] = nc.dram_tensor("do", [B, 3, 128, N], F32, kind="ExternalOutput")

    with tile.TileContext(nc) as tc:
        with (
            tc.tile_pool(name="sb", bufs=1) as pool,
            tc.tile_pool(name="ps", bufs=1, space="PSUM") as ps,
        ):
            ones16 = pool.tile([128, 64], F16, name="ones16")
            nc.vector.memset(ones16[:], 1.0)
            bias_sb = pool.tile([128, 6], F32, name="bias_sb")
            nc.sync.dma_start(out=bias_sb[:], in_=bias_d[:])

            proj_q = []
            ypc = [0]

            def emit_proj(drain=False):
                if not proj_q:
                    return
                t, n, OTq, wpq, boffq = proj_q.pop(0)
                tsl = slice(t * 128, (t + 1) * 128)
                nsl = slice(n * 512, (n + 1) * 512)
                tag = ["ypb", "bcb", "T0", "T1"][ypc[0] % 4] if drain else "ypb"
                ypc[0] += 1
                yp = ps.tile([128, 512], F32, tag=tag, name=f"yp{boffq}_{t}_{n}")
                nc.tensor.matmul(yp[:, 0:512], OTq[0][:, tsl], wpq[:, 0, nsl],
                                 start=True, stop=False)
                nc.tensor.matmul(yp[:, 0:512], OTq[1][:, tsl], wpq[:, 1, nsl],
                                 start=False, stop=False)
                nc.tensor.matmul(yp[:, 0:512], OTq[2][:, tsl], wpq[:, 2, nsl],
                                 start=False, stop=True)
                y_sb = pool.tile([128, 512], F32, tag="y", name=f"ysb{boffq}_{t}_{n}", bufs=3)
                nc.vector.tensor_copy(y_sb[:], yp[:, 0:512])
                nc.sync.dma_start(
                    out=y_d[boffq + t * 128: boffq + (t + 1) * 128, nsl], in_=y_sb[:])

            for b in range(B):
                boff = b * N

                # ---------------- phase Q: qkv projections ----------------
                scope_q = nc.enter_named_scope(f"phQ{b}", False)
                wall = pool.tile([128, KC, 960], F32R, tag="w", name=f"wall{b}")
                wall_r = wall_d.rearrange("(t p) f -> p t f", p=128)
                nc.sync.dma_start(out=wall[:, 0:KC // 2, :], in_=wall_r[:, 0:KC // 2, :])
                nc.sync.dma_start(out=wall[:, KC // 2:KC, :], in_=wall_r[:, KC // 2:KC, :])

                QT = [pool.tile([128, N], F16, tag=f"qt{i}", name=f"QT{i}_{b}") for i in range(3)]
                KT = [pool.tile([128, N], F16, tag=f"kt{i}", name=f"KT{i}_{b}") for i in range(3)]
                V = [pool.tile([128, HPC * 65], F16, tag=f"v{i}", name=f"V{i}_{b}")
                     for i in range(KT16)]
                V3 = [t.rearrange("p (h e) -> p h e", h=HPC) for t in V]
                for t3 in V3:
                    nc.vector.memset(t3[:, :, 64:65], 1.0)

                # NOTE: matmul start=True clears has_written for the WHOLE psum
                # bank, so each bank may hold only ONE accumulation group.
                for j in range(NCHUNK):
                    xb = pool.tile([128, KC, CK], F32R, tag=f"x{j % 2}", name=f"xb{b}_{j}")
                    nc.sync.dma_start(
                        out=xb[:],
                        in_=xT_d[:, boff + j * CK: boff + (j + 1) * CK]
                        .rearrange("(t p) c -> p t c", p=128))
                    cj = slice(j * CK, (j + 1) * CK)
                    tQ = ps.tile([128, 1024], F32, tag="T0", name=f"tQ{b}_{j}")
                    tK = ps.tile([128, 1024], F32, tag="T1", name=f"tK{b}_{j}")
                    tMa = ps.tile([128, 512], F32, tag="bcb", name=f"tMa{b}_{j}")
                    tMb = ps.tile([128, 512], F32, tag="ypb", name=f"tMb{b}_{j}")
                    tV = ps.tile([128, 1024], F32, tag="T2", name=f"tV{b}_{j}")
                    for K in range(KC):
                        st, sp = K == 0, K == KC - 1
                        nc.tensor.matmul(tQ[:, 0:256], wall[:, K, 0:128], xb[:, K, :],
                                         start=st, stop=sp)
                        nc.tensor.matmul(tQ[:, 512:768], wall[:, K, 128:256], xb[:, K, :],
                                         start=st, stop=sp)
                    with nc.allow_low_precision(reason="fp16 qkv"):
                        nc.vector.tensor_scalar_add(QT[0][:, cj], tQ[:, 0:256], bias_sb[:, 0:1])
                        nc.vector.tensor_scalar_add(QT[1][:, cj], tQ[:, 512:768], bias_sb[:, 1:2])
                    for K in range(KC):
                        st, sp = K == 0, K == KC - 1
                        nc.tensor.matmul(tK[:, 0:256], wall[:, K, 320:448], xb[:, K, :],
                                         start=st, stop=sp)
                        nc.tensor.matmul(tK[:, 512:768], wall[:, K, 448:576], xb[:, K, :],
                                         start=st, stop=sp)
                    with nc.allow_low_precision(reason="fp16 qkv"):
                        nc.vector.tensor_scalar_add(KT[0][:, cj], tK[:, 0:256], bias_sb[:, 3:4])
                        nc.vector.tensor_scalar_add(KT[1][:, cj], tK[:, 512:768], bias_sb[:, 4:5])
                    for K in range(KC):
                        st, sp = K == 0, K == KC - 1
                        nc.tensor.matmul(tMa[0:64, 0:256], wall[:, K, 256:320],
                                         xb[:, K, :], start=st, stop=sp, tile_position=(0, 0))
                        nc.tensor.matmul(tMb[0:64, 0:256], wall[:, K, 576:640],
                                         xb[:, K, :], start=st, stop=sp, tile_position=(0, 0))
                    with nc.allow_low_precision(reason="fp16 qkv"):
                        nc.vector.tensor_scalar_add(QT[2][0:64, cj], tMa[0:64, 0:256],
                                                    bias_sb[0:64, 2:3])
                        nc.vector.tensor_scalar_add(KT[2][0:64, cj], tMb[0:64, 0:256],
                                                    bias_sb[0:64, 5:6])
                    for K in range(KC):
                        st, sp = K == 0, K == KC - 1
                        nc.tensor.matmul(tV[:, 0:320], xb[:, K, 0:128], wall[:, K, 640:960],
                                         start=st, stop=sp)
                        nc.tensor.matmul(tV[:, 512:832], xb[:, K, 128:256], wall[:, K, 640:960],
                                         start=st, stop=sp)
                    with nc.allow_low_precision(reason="fp16 qkv"):
                        nc.vector.tensor_copy(
                            V3[2 * j][:, :, 0:64],
                            tV[:, 0:320].rearrange("p (h e) -> p h e", h=HPC))
                        nc.vector.tensor_copy(
                            V3[2 * j + 1][:, :, 0:64],
                            tV[:, 512:832].rearrange("p (h e) -> p h e", h=HPC))

                if DEBUG:
                    for i in range(3):
                        qf = pool.tile([128, N], F32, tag="dbg", name=f"dbq{b}_{i}")
                        nc.scalar.activation(qf[:], QT[i][:], AF.Copy)
                        nc.sync.dma_start(out=dbg["q"][b, i], in_=qf[:])
                        kf = pool.tile([128, N], F32, tag="dbg", name=f"dbk{b}_{i}")
                        nc.scalar.activation(kf[:], KT[i][:], AF.Copy)
                        nc.sync.dma_start(out=dbg["k"][b, i], in_=kf[:])
                    for i in range(KT16):
                        vf = pool.tile([128, N], F32, tag="dbg", name=f"dbv{b}_{i}")
                        nc.scalar.activation(vf[:, 0:HPC * 65], V[i][:], AF.Copy)
                        nc.sync.dma_start(out=dbg["v"][b, i], in_=vf[:, 0:HPC * 65])

                # ------------- phase A + P: attention with interleaved proj -------------
                nc.leave_named_scope(f"phQ{b}", scope_q[0], False)
                scope_a = nc.enter_named_scope(f"phA{b}", False)
                OT = [pool.tile([128, N], F16, tag=f"ot{i}", name=f"OT{i}_{b}") for i in range(3)]
                nc.vector.memset(OT[2][64:128, :], 0.0)
                wp = pool.tile([128, 3, C], F16, tag="wpt", name=f"wp{b}")
                nc.sync.dma_start(out=wp[:], in_=wp_d.rearrange("(g p) f -> p g f", p=128))


                OTb, wpb, boffb = OT, wp, boff

                def normalize(o_ps, r_t, rb_t, cols, dst, qbs, shift, nm):
                    """o_ps [0:65, 512]; divide rows 0:64 by denominator row 64
                    (reciprocal already computed into r_t)."""
                    bcb = ps.tile([128, 512], F32, tag="bcb", name=f"bc{nm}")
                    nc.tensor.matmul(bcb[0:64, :], ones16[64:65, :], r_t[64:65, cols],
                                     start=True, stop=True, tile_position=(64, 0))
                    nc.vector.tensor_copy(rb_t[0:64, cols], bcb[0:64, :])
                    if not shift:
                        with nc.allow_low_precision(reason="o f32r"):
                            nc.vector.tensor_mul(dst[0:64, qbs], o_ps[0:64, :], rb_t[0:64, cols])
                    else:
                        nc.vector.tensor_mul(bcb[64:128, :], o_ps[0:64, :], rb_t[0:64, cols])
                        with nc.allow_low_precision(reason="o f32r"):
                            nc.vector.tensor_copy(dst[64:128, qbs], bcb[64:128, :])

                pending = [None]

                def flush_pending():
                    if pending[0] is not None:
                        pending[0]()
                        pending[0] = None

                def unit_pair(p, qb):
                    """S/exp run one 2-kt step ahead of the O matmuls; the previous
                    unit's deferred normalize is emitted after step 1 so its bcast
                    matmul never head-blocks the PE queue."""
                    qt, kt_, qbs = QT[p], KT[p], slice(qb * QB, (qb + 1) * QB)
                    oAB = ps.tile([128, 1024], F32, tag="T2", name=f"oAB{b}_{p}_{qb}")

                    def s_step(k2):
                        pts = []
                        for kt in (2 * k2, 2 * k2 + 1):
                            s = ps.tile([128, 1024], F32, tag=f"T{kt % 2}",
                                        name=f"s{b}_{p}_{qb}_{kt}")
                            ksl = slice(kt * 128, (kt + 1) * 128)
                            nc.tensor.matmul(s[:, 0:512], kt_[0:64, ksl], qt[0:64, qbs],
                                             start=True, stop=True, tile_position=(0, 0))
                            nc.tensor.matmul(s[:, 512:1024], kt_[64:128, ksl], qt[64:128, qbs],
                                             start=True, stop=True, tile_position=(64, 0))
                            p_t = pool.tile([128, 1024], F16, tag="p",
                                            name=f"p{b}_{p}_{qb}_{kt}", bufs=5)
                            with nc.allow_low_precision(reason="fp16 probs"):
                                nc.scalar.activation(p_t[:], s[:], AF.Exp, scale=SCALE)
                            pts.append(p_t)
                        return pts

                    def o_step(k2, pts):
                        for i, kt in enumerate((2 * k2, 2 * k2 + 1)):
                            st, sp = kt == 0, kt == KT16 - 1
                            nc.tensor.matmul(oAB[0:65, 0:512], V3[kt][:, 2 * p, :],
                                             pts[i][:, 0:512], start=st, stop=sp)
                            nc.tensor.matmul(oAB[0:65, 512:1024], V3[kt][:, 2 * p + 1, :],
                                             pts[i][:, 512:1024], start=st, stop=sp)
                        emit_proj()

                    prev = s_step(0)
                    for k2 in range(1, KT16 // 2):
                        cur = s_step(k2)
                        if k2 == 1:
                            flush_pending()
                        o_step(k2 - 1, prev)
                        prev = cur
                    o_step(KT16 // 2 - 1, prev)
                    r_t = pool.tile([128, 1024], F16, tag="r", name=f"r{b}_{p}_{qb}", bufs=2)
                    with nc.allow_low_precision(reason="softmax recip"):
                        nc.vector.reciprocal(r_t[64:65, 0:512], oAB[64:65, 0:512])
                        nc.vector.reciprocal(r_t[64:65, 512:1024], oAB[64:65, 512:1024])

                    def _norm():
                        rb_t = pool.tile([128, 1024], F32, tag="rb", name=f"rb{b}_{p}_{qb}", bufs=2)
                        normalize(oAB[:, 0:512], r_t, rb_t, slice(0, 512), OT[p], qbs,
                                  False, f"{b}_{p}_{qb}a")
                        normalize(oAB[:, 512:1024], r_t, rb_t, slice(512, 1024), OT[p], qbs,
                                  True, f"{b}_{p}_{qb}b")
                    pending[0] = _norm

                def unit_lone(qb):
                    qbs = slice(qb * QB, (qb + 1) * QB)
                    oC = ps.tile([128, 1024], F32, tag="T2", name=f"oC{b}_{qb}")

                    def s_step(k2):
                        pts = []
                        for kt in (2 * k2, 2 * k2 + 1):
                            s = ps.tile([128, 1024], F32, tag=f"T{kt % 2}", name=f"sl{b}_{qb}_{kt}")
                            ksl = slice(kt * 128, (kt + 1) * 128)
                            nc.tensor.matmul(s[:, 0:512], KT[2][0:64, ksl], QT[2][0:64, qbs],
                                             start=True, stop=True, tile_position=(0, 0))
                            p_t = pool.tile([128, 1024], F16, tag="p",
                                            name=f"pl{b}_{qb}_{kt}", bufs=5)
                            with nc.allow_low_precision(reason="fp16 probs"):
                                nc.scalar.activation(p_t[:, 0:512], s[:, 0:512], AF.Exp, scale=SCALE)
                            pts.append(p_t)
                        return pts

                    def o_step(k2, pts):
                        for i, kt in enumerate((2 * k2, 2 * k2 + 1)):
                            st, sp = kt == 0, kt == KT16 - 1
                            nc.tensor.matmul(oC[0:65, 0:512], V3[kt][:, 4, :], pts[i][:, 0:512],
                                             start=st, stop=sp)
                        emit_proj()

                    prev = s_step(0)
                    for k2 in range(1, KT16 // 2):
                        cur = s_step(k2)
                        if k2 == 1:
                            flush_pending()
                        o_step(k2 - 1, prev)
                        prev = cur
                    o_step(KT16 // 2 - 1, prev)
                    r_t = pool.tile([128, 1024], F16, tag="r", name=f"rl{b}_{qb}", bufs=2)
                    with nc.allow_low_precision(reason="softmax recip"):
                        nc.vector.reciprocal(r_t[64:65, 0:512], oC[64:65, 0:512])

                    def _norm():
                        rb_t = pool.tile([128, 1024], F32, tag="rb", name=f"rbl{b}_{qb}", bufs=2)
                        normalize(oC[:, 0:512], r_t, rb_t, slice(0, 512), OT[2], qbs,
                                  False, f"{b}_l{qb}")
                    pending[0] = _norm

                for qb in range(NQB):
                    unit_pair(0, qb)
                    unit_pair(1, qb)
                    unit_lone(qb)
                    for t in range(qb * 4, qb * 4 + 4):
                        for n in range(5):
                            proj_q.append((t, n, OTb, wpb, boffb))
                flush_pending()

                if DEBUG:
                    for i in range(3):
                        of = pool.tile([128, N], F32, tag="dbg", name=f"dbo{b}_{i}")
                        nc.scalar.activation(of[:], OT[i][:], AF.Copy)
                        nc.sync.dma_start(out=dbg["o"][b, i], in_=of[:])
                nc.leave_named_scope(f"phA{b}", scope_a[0], False)
            while proj_q:
                emit_proj(drain=True)
    return nc


def kernel(x, w_qkv, b_qkv, w_proj, b_proj):
    x = np.asarray(x, np.float32)
    w_qkv = np.asarray(w_qkv, np.float32)
    b_qkv = np.asarray(b_qkv, np.float32)
    w_proj = np.asarray(w_proj, np.float32)
    b_proj = np.asarray(b_proj, np.float32)

    if "nc" not in _CACHE:
        nc = _build()
        nc.compile()
        _CACHE["nc"] = nc
    nc = _CACHE["nc"]

    xT = np.ascontiguousarray(x.reshape(TOK, C).T)            # [C, TOK]
    in_maps = []
    for c in range(NCORES):
        f0 = c * HPC * D                                       # 320*c
        qcols = slice(f0, f0 + HPC * D)
        wq = w_qkv[:, qcols]
        wk = w_qkv[:, C + f0: C + f0 + HPC * D]
        wv = w_qkv[:, 2 * C + f0: 2 * C + f0 + HPC * D]
        wall = np.concatenate([wq, wk, wv], axis=1)            # [C, 960]
        wp = np.zeros((384, C), np.float16)
        wp[0:320] = w_proj[f0:f0 + HPC * D, :]
        bias = np.zeros((128, 6), np.float32)
        bq = b_qkv[qcols]
        bk = b_qkv[C + f0: C + f0 + HPC * D]
        bias[:, 0] = bq[0:128]
        bias[:, 1] = bq[128:256]
        bias[0:64, 2] = bq[256:320]
        bias[:, 3] = bk[0:128]
        bias[:, 4] = bk[128:256]
        bias[0:64, 5] = bk[256:320]
        in_maps.append({"xT": xT, "wall": np.ascontiguousarray(wall),
                        "wp": wp, "bias": bias})

    _CACHE["in_maps"] = in_maps
    res = run_bass_kernel_spmd(nc, in_maps, core_ids=list(range(NCORES)))
    _CACHE["results"] = res.results
    y = np.zeros((TOK, C), np.float64)
    for c in range(NCORES):
        y += res.results[c]["y"].astype(np.float64)
    bias_eff = b_proj + b_qkv[2 * C:] @ w_proj                 # v-bias folded through proj
    y += bias_eff
    return y.reshape(B, N, C).astype(np.float32)


# revision 23
# speedup vs baseline: 1.0458x; 1.0366x over previous
"""Multi-head attention block (B=2, N=2048, C=2560, H=40, D=64) on 8 TRN2 NeuronCores.

Sharding: tensor-parallel over heads — core c owns heads 5c..5c+4 for both
batch elements. Each core computes qkv for its heads (full token range),
attention, and its partial contribution to the output projection; the host
sums the 8 partials and adds the (folded) output bias.

Per-core pipeline (per batch element, 2048 tokens):
  phase Q: stream xT in 256-token chunks; qkT matmuls (fp32r) produce
           Q^T/K^T feature-major tiles (fp16, +bias via DVE tensor_scalar);
           v matmuls produce V token-major tiles (fp16, with a ones column
           per head for the softmax denominator).
  phase A: per head-pair / q-block: S^T = k @ q^T via row-packed (tile_position)
           K=64 fp32->? fp16 matmuls; exp on ACT (scale folded in, no max
           subtraction needed at these magnitudes) -> P^T fp16; O^T and the
           denominator accumulate via [v|1] lhsT (M=65); normalization via
           DVE reciprocal + PE K=1 broadcast + DVE multiply (psum-shift for
           the odd head).
  phase P: y_partial = O^T.T @ w_proj (fp32r), streamed to DRAM.
"""
import numpy as np

import concourse.bacc as bacc
import concourse.mybir as mybir
import concourse.tile as tile
from concourse.bass_utils import run_bass_kernel_spmd

F32R = mybir.dt.float32r
F32 = mybir.dt.float32
F16 = mybir.dt.float16
AF = mybir.ActivationFunctionType

B, N, C = 2, 2048, 2560
H, D = 40, 64
NCORES = 8
HPC = H // NCORES            # 5 heads per core
SCALE = D ** -0.5
TOK = B * N                  # 4096
CK = 256                     # token chunk in phase Q
NCHUNK = N // CK             # 8 chunks per batch
KT16 = N // 128              # 16 k-tiles per batch
QB = 512                     # q-block
NQB = N // QB                # 4 q-blocks
KC = C // 128                # 20 contraction tiles

_CACHE = {}
DEBUG = False


def _build():
    nc = bacc.Bacc("TRN2", target_bir_lowering=False, debug=False, num_devices=NCORES)
    xT_d = nc.dram_tensor("xT", [C, TOK], F32R, kind="ExternalInput")
    wall_d = nc.dram_tensor("wall", [C, 960], F32R, kind="ExternalInput")   # q|k (640) + v (320)
    wp_d = nc.dram_tensor("wp", [384, C], F16, kind="ExternalInput")        # padded 320->384
    bias_d = nc.dram_tensor("bias", [128, 6], F32, kind="ExternalInput")    # per-ft qk bias
    y_d = nc.dram_tensor("y", [TOK, C], F32, kind="ExternalOutput")
    dbg = {}
    if DEBUG:
        dbg["q"] = nc.dram_tensor("dq", [B, 3, 128, N], F32, kind="ExternalOutput")
        dbg["k"] = nc.dram_tensor("dk", [B, 3, 128, N], F32, kind="ExternalOutput")
        dbg["v"] = nc.dram_tensor("dv", [B, KT16, 128, HPC * 65], F32, kind="ExternalOutput")
        dbg["o"] = nc.dram_tensor("do", [B, 3, 128, N], F32, kind="ExternalOutput")

    with tile.TileContext(nc) as tc:
        with (
            tc.tile_pool(name="sb", bufs=1) as pool,
            tc.tile_pool(name="ps", bufs=1, space="PSUM") as ps,
        ):
            ones16 = pool.tile([128, 64], F16, name="ones16")
            nc.vector.memset(ones16[:], 1.0)
            bias_sb = pool.tile([128, 6], F32, name="bias_sb")
            nc.sync.dma_start(out=bias_sb[:], in_=bias_d[:])

            proj_q = []
            ypc = [0]

            def emit_proj(drain=False):
                if not proj_q:
                    return
                t, n, OTq, wpq, boffq = proj_q.pop(0)
                tsl = slice(t * 128, (t + 1) * 128)
                nsl = slice(n * 512, (n + 1) * 512)
                tag = ["ypb", "bcb", "T0", "T1"][ypc[0] % 4] if drain else "ypb"
                ypc[0] += 1
                yp = ps.tile([128, 512], F32, tag=tag, name=f"yp{boffq}_{t}_{n}")
                nc.tensor.matmul(yp[:, 0:512], OTq[0][:, tsl], wpq[:, 0, nsl],
                                 start=True, stop=False)
                nc.tensor.matmul(yp[:, 0:512], OTq[1][:, tsl], wpq[:, 1, nsl],
                                 start=False, stop=False)
                nc.tensor.matmul(yp[:, 0:512], OTq[2][:, tsl], wpq[:, 2, nsl],
                                 start=False, stop=True)
                y_sb = pool.tile([128, 512], F32, tag="y", name=f"ysb{boffq}_{t}_{n}", bufs=3)
                nc.vector.tensor_copy(y_sb[:], yp[:, 0:512])
                nc.sync.dma_start(
                    out=y_d[boffq + t * 128: boffq + (t + 1) * 128, nsl], in_=y_sb[:])

            for b in range(B):
                boff = b * N

                # ---------------- phase Q: qkv projections ----------------
                scope_q = nc.enter_named_scope(f"phQ{b}", False)
                wall = pool.tile([128, KC, 960], F32R, tag="w", name=f"wall{b}")
                wall_r = wall_d.rearrange("(t p) f -> p t f", p=128)
                nc.sync.dma_start(out=wall[:, 0:KC // 2, :], in_=wall_r[:, 0:KC // 2, :])
                nc.sync.dma_start(out=wall[:, KC // 2:KC, :], in_=wall_r[:, KC // 2:KC, :])

                QT = [pool.tile([128, N], F16, tag=f"qt{i}", name=f"QT{i}_{b}") for i in range(3)]
                KT = [pool.tile([128, N], F16, tag=f"kt{i}", name=f"KT{i}_{b}") for i in range(3)]
                V = [pool.tile([128, HPC * 65], F16, tag=f"v{i}", name=f"V{i}_{b}")
                     for i in range(KT16)]
                V3 = [t.rearrange("p (h e) -> p h e", h=HPC) for t in V]
                for t3 in V3:
                    nc.vector.memset(t3[:, :, 64:65], 1.0)

                # NOTE: matmul start=True clears has_written for the WHOLE psum
                # bank, so each bank may hold only ONE accumulation group.
                for j in range(NCHUNK):
                    xb = pool.tile([128, KC, CK], F32R, tag=f"x{j % 2}", name=f"xb{b}_{j}")
                    nc.sync.dma_start(
                        out=xb[:],
                        in_=xT_d[:, boff + j * CK: boff + (j + 1) * CK]
                        .rearrange("(t p) c -> p t c", p=128))
                    cj = slice(j * CK, (j + 1) * CK)
                    tQ = ps.tile([128, 1024], F32, tag="T0", name=f"tQ{b}_{j}")
                    tK = ps.tile([128, 1024], F32, tag="T1", name=f"tK{b}_{j}")
                    tM = ps.tile([128, 512], F32, tag="bcb", name=f"tM{b}_{j}")
                    tMs = ps.tile([128, 512], F32, tag="ypb", name=f"tMs{b}_{j}")
                    tV = ps.tile([128, 1024], F32, tag="T2", name=f"tV{b}_{j}")
                    for K in range(KC):
                        st, sp = K == 0, K == KC - 1
                        nc.tensor.matmul(tQ[:, 0:256], wall[:, K, 0:128], xb[:, K, :],
                                         start=st, stop=sp)
                        nc.tensor.matmul(tQ[:, 512:768], wall[:, K, 128:256], xb[:, K, :],
                                         start=st, stop=sp)
                    with nc.allow_low_precision(reason="fp16 qkv"):
                        nc.vector.tensor_scalar_add(QT[0][:, cj], tQ[:, 0:256], bias_sb[:, 0:1])
                        nc.vector.tensor_scalar_add(QT[1][:, cj], tQ[:, 512:768], bias_sb[:, 1:2])
                    for K in range(KC):
                        st, sp = K == 0, K == KC - 1
                        nc.tensor.matmul(tK[:, 0:256], wall[:, K, 256:384], xb[:, K, :],
                                         start=st, stop=sp)
                        nc.tensor.matmul(tK[:, 512:768], wall[:, K, 384:512], xb[:, K, :],
                                         start=st, stop=sp)
                    with nc.allow_low_precision(reason="fp16 qkv"):
                        nc.vector.tensor_scalar_add(KT[0][:, cj], tK[:, 0:256], bias_sb[:, 3:4])
                        nc.vector.tensor_scalar_add(KT[1][:, cj], tK[:, 512:768], bias_sb[:, 4:5])
                    for K in range(KC):
                        st, sp = K == 0, K == KC - 1
                        nc.tensor.matmul(tM[:, 0:256], wall[:, K, 512:640],
                                         xb[:, K, :], start=st, stop=sp)
                    nc.vector.tensor_copy(tMs[64:128, 0:256], tM[0:64, 0:256])
                    with nc.allow_low_precision(reason="fp16 qkv"):
                        nc.vector.tensor_scalar_add(QT[2][64:128, cj], tMs[64:128, 0:256],
                                                    bias_sb[64:128, 2:3])
                        nc.vector.tensor_scalar_add(KT[2][64:128, cj], tM[64:128, 0:256],
                                                    bias_sb[64:128, 5:6])
                    for K in range(KC):
                        st, sp = K == 0, K == KC - 1
                        nc.tensor.matmul(tV[:, 0:320], xb[:, K, 0:128], wall[:, K, 640:960],
                                         start=st, stop=sp)
                        nc.tensor.matmul(tV[:, 512:832], xb[:, K, 128:256], wall[:, K, 640:960],
                                         start=st, stop=sp)
                    with nc.allow_low_precision(reason="fp16 qkv"):
                        nc.vector.tensor_copy(
                            V3[2 * j][:, :, 0:64],
                            tV[:, 0:320].rearrange("p (h e) -> p h e", h=HPC))
                        nc.vector.tensor_copy(
                            V3[2 * j + 1][:, :, 0:64],
                            tV[:, 512:832].rearrange("p (h e) -> p h e", h=HPC))

                if DEBUG:
                    for i in range(3):
                        qf = pool.tile([128, N], F32, tag="dbg", name=f"dbq{b}_{i}")
                        nc.scalar.activation(qf[:], QT[i][:], AF.Copy)
                        nc.sync.dma_start(out=dbg["q"][b, i], in_=qf[:])
                        kf = pool.tile([128, N], F32, tag="dbg", name=f"dbk{b}_{i}")
                        nc.scalar.activation(kf[:], KT[i][:], AF.Copy)
                        nc.sync.dma_start(out=dbg["k"][b, i], in_=kf[:])
                    for i in range(KT16):
                        vf = pool.tile([128, N], F32, tag="dbg", name=f"dbv{b}_{i}")
                        nc.scalar.activation(vf[:, 0:HPC * 65], V[i][:], AF.Copy)
                        nc.sync.dma_start(out=dbg["v"][b, i], in_=vf[:, 0:HPC * 65])

                # ------------- phase A + P: attention with interleaved proj -------------
                nc.leave_named_scope(f"phQ{b}", scope_q[0], False)
                scope_a = nc.enter_named_scope(f"phA{b}", False)
                OT = [pool.tile([128, N], F16, tag=f"ot{i}", name=f"OT{i}_{b}") for i in range(3)]
                nc.vector.memset(OT[2][64:128, :], 0.0)
                wp = pool.tile([128, 3, C], F16, tag="wpt", name=f"wp{b}")
                nc.sync.dma_start(out=wp[:], in_=wp_d.rearrange("(g p) f -> p g f", p=128))


                OTb, wpb, boffb = OT, wp, boff

                def normalize(o_ps, r_t, rb_t, cols, dst, qbs, shift, nm):
                    """o_ps [0:65, 512]; divide rows 0:64 by denominator row 64
                    (reciprocal already computed into r_t)."""
                    bcb = ps.tile([128, 512], F32, tag="bcb", name=f"bc{nm}")
                    nc.tensor.matmul(bcb[0:64, :], ones16[64:65, :], r_t[64:65, cols],
                                     start=True, stop=True, tile_position=(64, 0))
                    nc.vector.tensor_copy(rb_t[0:64, cols], bcb[0:64, :])
                    if not shift:
                        with nc.allow_low_precision(reason="o f32r"):
                            nc.vector.tensor_mul(dst[0:64, qbs], o_ps[0:64, :], rb_t[0:64, cols])
                    else:
                        nc.vector.tensor_mul(bcb[64:128, :], o_ps[0:64, :], rb_t[0:64, cols])
                        with nc.allow_low_precision(reason="o f32r"):
                            nc.vector.tensor_copy(dst[64:128, qbs], bcb[64:128, :])

                pending = [None]

                def flush_pending():
                    if pending[0] is not None:
                        pending[0]()
                        pending[0] = None

                def unit_pair(p, qb):
                    """S/exp run one 2-kt step ahead of the O matmuls; the previous
                    unit's deferred normalize is emitted after step 1 so its bcast
                    matmul never head-blocks the PE queue."""
                    qt, kt_, qbs = QT[p], KT[p], slice(qb * QB, (qb + 1) * QB)
                    oAB = ps.tile([128, 1024], F32, tag="T2", name=f"oAB{b}_{p}_{qb}")

                    def s_step(k2):
                        pts = []
                        for kt in (2 * k2, 2 * k2 + 1):
                            s = ps.tile([128, 1024], F32, tag=f"T{kt % 2}",
                                        name=f"s{b}_{p}_{qb}_{kt}")
                            ksl = slice(kt * 128, (kt + 1) * 128)
                            nc.tensor.matmul(s[:, 0:512], kt_[0:64, ksl], qt[0:64, qbs],
                                             start=True, stop=True, tile_position=(0, 0))
                            nc.tensor.matmul(s[:, 512:1024], kt_[64:128, ksl], qt[64:128, qbs],
                                             start=True, stop=True, tile_position=(64, 0))
                            p_t = pool.tile([128, 1024], F16, tag="p",
                                            name=f"p{b}_{p}_{qb}_{kt}", bufs=4)
                            with nc.allow_low_precision(reason="fp16 probs"):
                                nc.scalar.activation(p_t[:], s[:], AF.Exp, scale=SCALE)
                            pts.append(p_t)
                        return pts

                    def o_step(k2, pts):
                        for i, kt in enumerate((2 * k2, 2 * k2 + 1)):
                            st, sp = kt == 0, kt == KT16 - 1
                            nc.tensor.matmul(oAB[0:65, 0:512], V3[kt][:, 2 * p, :],
                                             pts[i][:, 0:512], start=st, stop=sp)
                            nc.tensor.matmul(oAB[0:65, 512:1024], V3[kt][:, 2 * p + 1, :],
                                             pts[i][:, 512:1024], start=st, stop=sp)
                        emit_proj()

                    prev = s_step(0)
                    for k2 in range(1, KT16 // 2):
                        cur = s_step(k2)
                        if k2 == 1:
                            flush_pending()
                        o_step(k2 - 1, prev)
                        prev = cur
                    o_step(KT16 // 2 - 1, prev)
                    r_t = pool.tile([128, 1024], F16, tag="r", name=f"r{b}_{p}_{qb}", bufs=2)
                    with nc.allow_low_precision(reason="softmax recip"):
                        nc.vector.reciprocal(r_t[64:65, 0:512], oAB[64:65, 0:512])
                        nc.vector.reciprocal(r_t[64:65, 512:1024], oAB[64:65, 512:1024])

                    def _norm():
                        rb_t = pool.tile([128, 1024], F32, tag="rb", name=f"rb{b}_{p}_{qb}", bufs=2)
                        normalize(oAB[:, 0:512], r_t, rb_t, slice(0, 512), OT[p], qbs,
                                  False, f"{b}_{p}_{qb}a")
                        normalize(oAB[:, 512:1024], r_t, rb_t, slice(512, 1024), OT[p], qbs,
                                  True, f"{b}_{p}_{qb}b")
                    pending[0] = _norm

                def unit_lone(qb):
                    qbs = slice(qb * QB, (qb + 1) * QB)
                    oC = ps.tile([128, 1024], F32, tag="T2", name=f"oC{b}_{qb}")

                    def s_step(k2):
                        pts = []
                        for kt in (2 * k2, 2 * k2 + 1):
                            s = ps.tile([128, 1024], F32, tag=f"T{kt % 2}", name=f"sl{b}_{qb}_{kt}")
                            ksl = slice(kt * 128, (kt + 1) * 128)
                            nc.tensor.matmul(s[:, 0:512], KT[2][64:128, ksl], QT[2][64:128, qbs],
                                             start=True, stop=True, tile_position=(64, 0))
                            p_t = pool.tile([128, 1024], F16, tag="p",
                                            name=f"pl{b}_{qb}_{kt}", bufs=4)
                            with nc.allow_low_precision(reason="fp16 probs"):
                                nc.scalar.activation(p_t[:, 0:512], s[:, 0:512], AF.Exp, scale=SCALE)
                            pts.append(p_t)
                        return pts

                    def o_step(k2, pts):
                        for i, kt in enumerate((2 * k2, 2 * k2 + 1)):
                            st, sp = kt == 0, kt == KT16 - 1
                            nc.tensor.matmul(oC[0:65, 0:512], V3[kt][:, 4, :], pts[i][:, 0:512],
                                             start=st, stop=sp)
                        emit_proj()

                    prev = s_step(0)
                    for k2 in range(1, KT16 // 2):
                        cur = s_step(k2)
                        if k2 == 1:
                            flush_pending()
                        o_step(k2 - 1, prev)
                        prev = cur
                    o_step(KT16 // 2 - 1, prev)
                    r_t = pool.tile([128, 1024], F16, tag="r", name=f"rl{b}_{qb}", bufs=2)
                    with nc.allow_low_precision(reason="softmax recip"):
                        nc.vector.reciprocal(r_t[64:65, 0:512], oC[64:65, 0:512])

                    def _norm():
                        rb_t = pool.tile([128, 1024], F32, tag="rb", name=f"rbl{b}_{qb}", bufs=2)
                        normalize(oC[:, 0:512], r_t, rb_t, slice(0, 512), OT[2], qbs,
                                  False, f"{b}_l{qb}")
                    pending[0] = _norm

                for qb in range(NQB):
                    unit_pair(0, qb)
                    unit_pair(1, qb)
                    unit_lone(qb)
                    for t in range(qb * 4, qb * 4 + 4):
                        for n in range(5):
                            proj_q.append((t, n, OTb, wpb, boffb))
                flush_pending()

                if DEBUG:
                    for i in range(3):
                        of = pool.tile([128, N], F32, tag="dbg", name=f"dbo{b}_{i}")
                        nc.scalar.activation(of[:], OT[i][:], AF.Copy)
                        nc.sync.dma_start(out=dbg["o"][b, i], in_=of[:])
                nc.leave_named_scope(f"phA{b}", scope_a[0], False)
            while proj_q:
                emit_proj(drain=True)
    return nc


def kernel(x, w_qkv, b_qkv, w_proj, b_proj):
    x = np.asarray(x, np.float32)
    w_qkv = np.asarray(w_qkv, np.float32)
    b_qkv = np.asarray(b_qkv, np.float32)
    w_proj = np.asarray(w_proj, np.float32)
    b_proj = np.asarray(b_proj, np.float32)

    if "nc" not in _CACHE:
        nc = _build()
        nc.compile()
        _CACHE["nc"] = nc
    nc = _CACHE["nc"]

    xT = np.ascontiguousarray(x.reshape(TOK, C).T)            # [C, TOK]
    in_maps = []
    for c in range(NCORES):
        f0 = c * HPC * D                                       # 320*c
        qcols = slice(f0, f0 + HPC * D)
        wq = w_qkv[:, qcols]
        wk = w_qkv[:, C + f0: C + f0 + HPC * D]
        wv = w_qkv[:, 2 * C + f0: 2 * C + f0 + HPC * D]
        # [q0..q3 (256) | k0..k3 (256) | q4|k4 (128) | v (320)]
        wall = np.concatenate([wq[:, 0:256], wk[:, 0:256],
                               wq[:, 256:320], wk[:, 256:320], wv], axis=1)
        wp = np.zeros((384, C), np.float16)
        wp[0:320] = w_proj[f0:f0 + HPC * D, :]
        bias = np.zeros((128, 6), np.float32)
        bq = b_qkv[qcols]
        bk = b_qkv[C + f0: C + f0 + HPC * D]
        bias[:, 0] = bq[0:128]
        bias[:, 1] = bq[128:256]
        bias[64:128, 2] = bq[256:320]
        bias[:, 3] = bk[0:128]
        bias[:, 4] = bk[128:256]
        bias[64:128, 5] = bk[256:320]
        in_maps.append({"xT": xT, "wall": np.ascontiguousarray(wall),
                        "wp": wp, "bias": bias})

    _CACHE["in_maps"] = in_maps
    res = run_bass_kernel_spmd(nc, in_maps, core_ids=list(range(NCORES)))
    _CACHE["results"] = res.results
    y = np.zeros((TOK, C), np.float64)
    for c in range(NCORES):
        y += res.results[c]["y"].astype(np.float64)
    bias_eff = b_proj + b_qkv[2 * C:] @ w_proj                 # v-bias folded through proj
    y += bias_eff
    return y.reshape(B, N, C).astype(np.float32)
